# revision 53
# baseline (speedup 1.0000x reference)
"""Trainium2 Bass kernel for the DSCNMP GNN (2x GINConv + pooling + MLP head).

Self-contained: takes full (unsharded) inputs, shards nodes/edges across the
8 NeuronCores internally, runs one SPMD Bass program via
bass_utils.run_bass_kernel_spmd, and returns the full [G, O] output.

Sharding strategy (per the problem's hint):
  - Nodes partitioned contiguously across 8 cores; each edge owned by the
    core of its dst node. Small MLP/BN weights replicated.
  - Per-edge gathers via dma_gather from padded node tables in DRAM
    (4 chunks to satisfy the int16 index range).
  - Segment-sum on the TensorEngine: per 128-edge slot, a one-hot selection
    matrix S (S[e, j] = dst_e == j) is built on DVE and agg^T accumulates in
    PSUM as land^T @ S per 128-node tile (dma_scatter_add loses duplicate
    updates on HW, so scatter is not used).
  - x1 node features AllGathered (bf16) so every core can gather remote rows.
  - Pooled graph embeddings AllReduced; graph-level head replicated.

All floating-point math happens on device; the host does integer index and
layout preprocessing only.
"""

import numpy as np

N_FULL, E_FULL, G_FULL, C_DIM, H_DIM, O_DIM = 100000, 600000, 1000, 2, 128, 10
HC_DIM = H_DIM // 2
NCORES = 8
NCHUNK = 4          # int16 gather-index chunking of the global table
TGRP = 8            # node tiles per gather group
EPS = 1e-5

_CACHE = {}


def _pack_idx16(flat):
    """[j%16, j//16] int16 packing, replicated across the 8 Q7 groups."""
    total = len(flat)
    assert total % 16 == 0
    out = flat.reshape(total // 16, 16).T.astype(np.int16)
    return np.tile(out, (8, 1))


def _preprocess(pos, edge_index, batch, N, E, G):
    NL = N // NCORES
    NLP = -(-NL // 512) * 512
    NT = NLP // 128
    NGRP = -(-NT // TGRP)
    NTP = NGRP * TGRP
    TOT = NLP * NCORES
    QL = NLP // NCHUNK
    CHUNK = QL * NCORES          # rows per quarter-table
    assert CHUNK <= 32767 and QL % 128 == 0

    pos = np.asarray(pos, np.float32)
    src = np.asarray(edge_index[0], np.int64)
    dst = np.asarray(edge_index[1], np.int64)
    batch = np.asarray(batch, np.int64)
    assert N % NCORES == 0

    node = np.arange(N)
    slot_of = NLP * (node // NL) + (node % NL)

    posT = np.zeros((NCORES, C_DIM, NLP), np.float32)   # only for in_maps ref
    pos_nm = np.zeros((NCORES, 128, NT * C_DIM), np.float32)
    batch_rel = np.full((NCORES, 128, NT), -5.0, np.float32)
    g0 = np.zeros(NCORES, np.int64)
    gwin_need = 0
    for k in range(NCORES):
        nodes = np.arange(k * NL, (k + 1) * NL)
        j = nodes - k * NL
        posT[k][:, j] = pos[nodes].T
        pos_nm[k][j % 128, (j // 128) * C_DIM + 0] = pos[nodes, 0]
        pos_nm[k][j % 128, (j // 128) * C_DIM + 1] = pos[nodes, 1]
        g0[k] = batch[nodes[0]]
        rel = batch[nodes] - g0[k]
        batch_rel[k][j % 128, j // 128] = rel.astype(np.float32)
        gwin_need = max(gwin_need, int(rel.max()) + 1)
    GWIN = min(512, max(128, -(-gwin_need // 32) * 32))
    assert gwin_need <= GWIN <= 512
    WG = -(-(G + GWIN) // 256) * 256


    # ---- edge cells: (core, quarter-table, node-tile) ----
    ecore = dst // NL
    ksrc = slot_of[src] // NLP
    jsrc = slot_of[src] % NLP
    grow_q = ksrc * QL + (jsrc % QL)     # row within quarter-table
    qsrc = jsrc // QL
    dslot = slot_of[dst] % NLP
    cells = [[None] * (NCHUNK * NT) for _ in range(NCORES)]
    capc = 128
    for k in range(NCORES):
        m = ecore == k
        gs, ds_ = grow_q[m], dslot[m]
        ch = qsrc[m]
        tt = ds_ // 128
        for c in range(NCHUNK):
            for t in range(NT):
                mm = (ch == c) & (tt == t)
                gg, dd = gs[mm], ds_[mm]
                o = np.argsort(dd, kind="stable")
                cells[k][c * NT + t] = (gg[o], dd[o] - t * 128)
                capc = max(capc, -(-len(gg) // 128) * 128)
    CAPC = capc
    SLC = CAPC // 128                     # slots per cell
    STREAM = NGRP * NCHUNK * TGRP * CAPC  # total padded positions
    NSLOT = STREAM // 128

    gidx2 = np.zeros((NCORES, 128, STREAM // 16), np.int16)
    dwc = np.full((NCORES, 128, NSLOT), -5.0, np.float32)
    dwc2 = np.full((NCORES, 128, NSLOT), -5.0, np.float32)
    posE = np.zeros((NCORES, 128, NSLOT * C_DIM), np.float32)
    invq = np.full((NCHUNK, CHUNK), -1, np.int64)
    invq[qsrc * 0 + 0, 0] = 0  # placeholder shape init
    invq = np.full((NCHUNK, CHUNK), 0, np.int64)
    allk = slot_of // NLP
    allj = slot_of % NLP
    invq[allj // QL, allk * QL + (allj % QL)] = np.arange(N)
    for k in range(NCORES):
        gi2 = np.zeros(STREAM, np.int64)
        dw = np.full(STREAM, -5.0, np.float32)
        dw2 = np.full(STREAM, -5.0, np.float32)
        pe = np.zeros((STREAM, C_DIM), np.float32)
        for g in range(NGRP):
            for c in range(NCHUNK):
                for tl in range(TGRP):
                    t = g * TGRP + tl
                    base = ((g * NCHUNK + c) * TGRP + tl) * CAPC
                    base2 = ((c * NGRP + g) * TGRP + tl) * CAPC
                    if t < NT:
                        gg, dd = cells[k][c * NT + t]
                        gi2[base2:base2 + len(gg)] = gg
                        dw[base:base + len(dd)] = dd.astype(np.float32)
                        dw2[base2:base2 + len(dd)] = dd.astype(np.float32)
                        pe[base:base + len(gg)] = pos[invq[c, gg]]
        gidx2[k] = _pack_idx16(gi2)
        dwc[k] = dw.reshape(NSLOT, 128).T
        dwc2[k] = dw2.reshape(NSLOT, 128).T
        posE[k] = pe.reshape(NSLOT, 128, C_DIM).transpose(1, 0, 2).reshape(
            128, NSLOT * C_DIM)
    groff = np.zeros((NCORES, 1, 2), np.int32)
    groff[:, 0, 0] = g0
    assert (g0 + GWIN <= WG).all()

    dims = dict(N=N, E=E, G=G, NL=NL, NLP=NLP, NT=NT, NGRP=NGRP, NTP=NTP,
                TOT=TOT, CHUNK=CHUNK, CAPC=CAPC, SLC=SLC, STREAM=STREAM,
                GWIN=GWIN, WG=WG)
    arrays = dict(posE=posE, pos_nm=pos_nm, batch_rel=batch_rel,
                  gidx2=gidx2, dwc=dwc, dwc2=dwc2, groff=groff)
    return dims, arrays


def _build_program(dims):
    import contextlib
    import concourse.bass as bass
    import concourse.bacc as bacc
    import concourse.mybir as mybir
    import concourse.tile as tile
    from concourse import library_config
    from concourse.masks import make_identity

    f32 = mybir.dt.float32
    bf16 = mybir.dt.bfloat16
    i16 = mybir.dt.int16
    i32 = mybir.dt.int32
    AF = mybir.ActivationFunctionType
    ALU = mybir.AluOpType

    NLP, NT, NGRP = dims["NLP"], dims["NT"], dims["NGRP"]
    QL = NLP // NCHUNK
    TOT, CHUNK, CAPC, SLC = dims["TOT"], dims["CHUNK"], dims["CAPC"], dims["SLC"]
    STREAM, GWIN, WG, G = dims["STREAM"], dims["GWIN"], dims["WG"], dims["G"]
    CALL = TGRP * CAPC              # idxs per dma_gather call
    WSZ = next(w for w in (512, 384, 256, 128) if NLP % w == 0)
    NW = NLP // WSZ                 # windows for MLP sweeps

    nc = bacc.Bacc("TRN2", target_bir_lowering=False, debug=False,
                   enable_asserts=True, num_devices=NCORES,
                   num_swdge_queues=4)

    def din(name, shape, dt=f32):
        return nc.dram_tensor(name, list(shape), dt, kind="ExternalInput")

    posE_d = din("posE", [128, (STREAM // 128) * C_DIM], bf16)
    pos_nm_d = din("pos_nm", [128, NT * C_DIM], bf16)
    batch_rel_d = din("batch_rel", [128, NT])
    gidx2_d = din("gidx2", [128, STREAM // 16], i16)
    dwc_d = din("dwc", [128, STREAM // 128])
    dwc2_d = din("dwc2", [128, STREAM // 128])
    groff_d = din("groff", [1, 2], i32)
    iota_d = din("iota", [128, max(GWIN, 128)], bf16)

    wnames = {}
    for nm, shp in [("W1a", [C_DIM, H_DIM]), ("W1b", [H_DIM, H_DIM]),
                    ("W2a", [H_DIM, H_DIM]), ("W2b", [H_DIM, H_DIM]),
                    ("Wf1", [C_DIM, H_DIM]), ("Wf2", [H_DIM, H_DIM]),
                    ("Wc1", [H_DIM, HC_DIM]), ("Wc2", [HC_DIM, O_DIM])]:
        wnames[nm] = din(nm, shp)
    vecs = {}
    for nm in ["b1a", "b1b", "b2a", "b2b", "bf1", "bf2",
               "n1_g", "n1_b", "n1_rm", "n1_rv", "n2_g", "n2_b", "n2_rm", "n2_rv",
               "f1_g", "f1_b", "f1_rm", "f1_rv", "f2_g", "f2_b", "f2_rm", "f2_rv"]:
        vecs[nm] = din(nm, [H_DIM, 1])
    for nm in ["bc1", "gc", "bec", "rmc", "rvc", "a_prelu_v"]:
        vecs[nm] = din(nm, [HC_DIM, 1])
    vecs["bc2"] = din("bc2", [O_DIM, 1])

    out_d = nc.dram_tensor("out", [G, O_DIM], f32, kind="ExternalOutput")

    with tile.TileContext(nc) as tc:
        nc.gpsimd.load_library(library_config.mlp)
        ctx = contextlib.ExitStack()
        with ctx:
            dram = ctx.enter_context(tc.tile_pool(name="dram", bufs=1, space="DRAM"))
            pconst = ctx.enter_context(tc.tile_pool(name="const", bufs=1))
            pbig = ctx.enter_context(tc.tile_pool(name="big", bufs=1))
            pland = ctx.enter_context(tc.tile_pool(name="land", bufs=8))
            psmall = ctx.enter_context(tc.tile_pool(name="small", bufs=4))
            pgr = ctx.enter_context(tc.tile_pool(name="gr", bufs=1))
            ph1 = ctx.enter_context(tc.tile_pool(name="h1w", bufs=2))
            ppsum = ctx.enter_context(tc.tile_pool(name="psum", bufs=2, space="PSUM"))
            pseg = ctx.enter_context(tc.tile_pool(name="psum_seg", bufs=4, space="PSUM"))
            ppool = ctx.enter_context(tc.tile_pool(name="psum_acc", bufs=2, space="PSUM"))

            cc_in = [dram.tile([QL, H_DIM], bf16, tag="cc_in", name=f"cc_in{q}",
                                bufs=NCHUNK) for q in range(NCHUNK)]
            cc_out = [dram.tile([QL * NCORES, H_DIM], bf16, tag="cc_out",
                                name=f"cc_out{q}", addr_space="Shared",
                                bufs=NCHUNK) for q in range(NCHUNK)]
            ar1_in = dram.tile([H_DIM + C_DIM, WG], bf16, tag="ar1_in")
            ar1_out = dram.tile([H_DIM + C_DIM, WG], bf16, tag="ar1_out", addr_space="Shared")
            ar2_in = dram.tile([H_DIM, WG], bf16, tag="ar2_in")
            ar2_out = dram.tile([H_DIM, WG], bf16, tag="ar2_out", addr_space="Shared")

            _ld_engines = [nc.sync, nc.scalar]
            _ld_state = {"i": 0}

            def load_const(dr, shape, dt=f32):
                t = pconst.tile(shape, dt, tag=dr.name + "_sb")
                eng = _ld_engines[_ld_state["i"] % len(_ld_engines)]
                _ld_state["i"] += 1
                eng.dma_start(out=t[:], in_=dr.ap())
                return t

            W = {k: load_const(v, v.shape) for k, v in wnames.items()}
            V = {k: load_const(v, v.shape) for k, v in vecs.items()}
            pos_nm = load_const(pos_nm_d, [128, NT * C_DIM], bf16)
            posE = load_const(posE_d, [128, (STREAM // 128) * C_DIM], bf16)
            batch_rel = load_const(batch_rel_d, [128, NT])
            iota_bf = load_const(iota_d, [128, max(GWIN, 128)], bf16)
            gidx2 = load_const(gidx2_d, [128, STREAM // 16], i16)
            dwc = load_const(dwc_d, [128, STREAM // 128])
            dwc2 = load_const(dwc2_d, [128, STREAM // 128])
            groff = load_const(groff_d, [1, 2], i32)

            ident = pconst.tile([128, 128], f32, tag="ident")
            make_identity(nc, ident[:])

            def bn_vec(g, b, rm, rv, P, nm):
                a = pconst.tile([P, 1], f32, tag=f"bn_a_{nm}")
                c = pconst.tile([P, 1], f32, tag=f"bn_c_{nm}")
                nc.vector.tensor_scalar(a[:], rv[:], EPS, None, ALU.add)
                nc.scalar.activation(a[:], a[:], AF.Sqrt)
                nc.vector.reciprocal(a[:], a[:])
                nc.vector.tensor_tensor(a[:], a[:], g[:], op=ALU.mult)
                nc.vector.tensor_tensor(c[:], rm[:], a[:], op=ALU.mult)
                nc.vector.tensor_tensor(c[:], b[:], c[:], op=ALU.subtract)
                return a, c
            a1, c1 = bn_vec(V["n1_g"], V["n1_b"], V["n1_rm"], V["n1_rv"], H_DIM, "n1")
            a2, c2 = bn_vec(V["n2_g"], V["n2_b"], V["n2_rm"], V["n2_rv"], H_DIM, "n2")
            af1, cf1 = bn_vec(V["f1_g"], V["f1_b"], V["f1_rm"], V["f1_rv"], H_DIM, "f1")
            af2, cf2 = bn_vec(V["f2_g"], V["f2_b"], V["f2_rm"], V["f2_rv"], H_DIM, "f2")
            acl, ccl = bn_vec(V["gc"], V["bec"], V["rmc"], V["rvc"], HC_DIM, "cls")

            # persistent big buffers
            xT = pbig.tile([128, NLP], bf16, tag="B")        # x1T -> h2T -> (x2T)
            xnm = pbig.tile([128, NT * H_DIM], bf16, tag="NM")  # x1nm then x2nm

            # ---------------- segment-sum sweep (shared structure) ----------
            # S matrices for 8 consecutive 128-edge slots are built in ONE
            # DVE op: S8[p, s*128+j] = (iota128[j] == dwc[p, s0+s]) via
            # step-0 broadcast APs on both operands. S8s only depend on the
            # static dwc table, so they are prebuilt one group ahead (bufs=12
            # on the S8 tag) to keep the PE fed.
            NSLOT_T = STREAM // 128
            NS8 = -(-NSLOT_T // 8)

            def build_s8(dwc_t, s0, nsl):
                S8 = psmall.tile([128, 8 * 128], bf16, tag="S8", bufs=12)
                nc.vector.tensor_tensor(
                    out=S8[:, 0:nsl * 128].rearrange("p (s j) -> p s j", j=128),
                    in0=iota_bf[:, None, 0:128].to_broadcast([128, nsl, 128]),
                    in1=dwc_t[:, s0:s0 + nsl, None].to_broadcast(
                        [128, nsl, 128]),
                    op=ALU.is_equal)
                return S8

            def seg_sweep(flush_fn, lhs_cols, post_mm, land_fn, post_group):
                s8tiles = {}

                def build8(g):
                    for k in range(8):
                        s8i = g * 8 + k
                        if s8i >= NS8:
                            return
                        ns8 = min(8, NSLOT_T - s8i * 8)
                        s8tiles[s8i] = build_s8(dwc, s8i * 8, ns8)

                build8(0)
                for g in range(NGRP):
                    if g + 1 < NGRP:
                        build8(g + 1)
                    lands = [land_fn(g, c) for c in range(NCHUNK)]
                    for tl in range(TGRP):
                        t = g * TGRP + tl
                        if t >= NT:
                            break
                        ps = pseg.tile([lhs_cols, 128], f32, tag="seg")
                        first = True
                        for c in range(NCHUNK):
                            for sl in range(SLC):
                                slot = ((g * NCHUNK + c) * TGRP + tl) * SLC + sl
                                S = s8tiles[slot // 8]
                                soff = (slot % 8) * 128
                                nc.tensor.matmul(
                                    ps[:], lands[c](tl, sl),
                                    S[:, soff:soff + 128], start=first,
                                    stop=False)
                                first = False
                        post_mm(t, ps)
                        flush_fn(t, ps)
                    for k in range(8):
                        s8tiles.pop(g * 8 + k, None)
                    post_group(g)

            # ================= conv1 =================
            # psum[t] rows 0:2 = agg(pos)^T tile; pos^T added via matmul vs
            # identity (lhsT dtype must match rhs -> bf16 identity).
            ident_bf = pconst.tile([128, 128], bf16, tag="ident_bf")
            nc.vector.tensor_copy(ident_bf[:], ident[:])
            W1b_bf = pconst.tile([H_DIM, H_DIM], bf16, tag="W1b_bf")
            nc.vector.tensor_copy(W1b_bf[:], W["W1b"][:])
            W2a_bf = pconst.tile([H_DIM, H_DIM], bf16, tag="W2a_bf")
            nc.vector.tensor_copy(W2a_bf[:], W["W2a"][:])
            W2b_bf = pconst.tile([H_DIM, H_DIM], bf16, tag="W2b_bf")
            nc.vector.tensor_copy(W2b_bf[:], W["W2b"][:])
            W1a_bf = pconst.tile([C_DIM, H_DIM], bf16, tag="W1a_bf")
            nc.vector.tensor_copy(W1a_bf[:], W["W1a"][:])
            h1w = {}

            def seg1_post(t, ps):
                nc.tensor.matmul(ps[:], pos_nm[:, t * C_DIM:(t + 1) * C_DIM],
                                 ident_bf[:], start=False, stop=True)

            def flush1(t, ps):
                w, sub = divmod(t * 128, WSZ)
                if w not in h1w:
                    h1w[w] = ph1.tile([C_DIM, WSZ], bf16, tag="h1w",
                                      name=f"h1w{w}", bufs=4)
                nc.scalar.copy(h1w[w][:, sub:sub + 128], ps[0:C_DIM, :])

            posE_v = posE[:].rearrange("p (s c) -> p s c", c=C_DIM)

            def land1(g, c):
                def get(tl, sl):
                    slot = ((g * NCHUNK + c) * TGRP + tl) * SLC + sl
                    return posE_v[:, slot, :]
                return get

            # conv1 MLP + transpose for one window (4 node tiles), then
            # AllGather a chunk of the x1 table as soon as it is complete —
            # the 4 AllGathers hide under the rest of the conv1 sweep.
            QT = QL // 128
            TPW = WSZ // 128  # tiles per MLP window

            def cc_dma(q):
                nc.sync.dma_start(
                    out=cc_in[q][:].rearrange("(s p) f -> p s f", p=128),
                    in_=xnm[:, q * QT * H_DIM:(q + 1) * QT * H_DIM].rearrange(
                        "p (s f) -> p s f", f=H_DIM))

            def cc_trig(q):
                nc.gpsimd.collective_compute(
                    "AllGather", mybir.AluOpType.bypass,
                    ins=[cc_in[q].opt()], outs=[cc_out[q].opt()],
                    replica_groups=[list(range(NCORES))])

            # zero-fill the AllReduce scratch windows up front
            zrow = pgr.tile([H_DIM, 256], bf16, tag="zrow")
            nc.vector.memset(zrow[:], 0.0)
            for zc in range(0, WG, 256):
                nc.sync.dma_start(out=ar1_in[0:H_DIM, zc:zc + 256], in_=zrow[:])
                nc.sync.dma_start(out=ar1_in[H_DIM:, zc:zc + 256],
                                  in_=zrow[0:C_DIM, :])
                nc.sync.dma_start(out=ar2_in[:, zc:zc + 256], in_=zrow[:])

            ps_pos_t = ppool.tile([C_DIM, GWIN], f32, tag="acc")
            ps_x1_t = ppool.tile([128, GWIN], f32, tag="acc")
            ps_pos = ps_pos_t[:]
            ps_x1 = ps_x1_t[:]

            def pool_tile(s, psacc, table, cols, first, last):
                B = psmall.tile([128, GWIN], bf16, tag="B")
                nc.vector.tensor_scalar(B[:], iota_bf[:, 0:GWIN],
                                        batch_rel[:, s:s + 1], None,
                                        ALU.is_equal)
                nc.tensor.matmul(psacc, table[:, s * cols:(s + 1) * cols],
                                 B[:], start=first, stop=last)
                return B

            def conv1_win(w):
                c0 = w * WSZ
                zwin = psmall.tile([128, WSZ], bf16, tag="zwin", bufs=2)
                ps = ppsum.tile([H_DIM, WSZ], f32, tag="work")
                nc.tensor.matmul(ps[:], W1a_bf[:], h1w[w][:],
                                 start=True, stop=True)
                nc.scalar.activation(zwin[:], ps[:], AF.Relu,
                                     bias=V["b1a"][:], scale=1.0)
                ps2 = ppsum.tile([H_DIM, WSZ], f32, tag="work")
                nc.tensor.matmul(ps2[:], W1b_bf[:], zwin[:],
                                 start=True, stop=True)
                nc.scalar.activation(ps2[:], ps2[:], AF.Relu,
                                     bias=V["b1b"][:], scale=1.0)
                nc.vector.tensor_scalar(xT[:, c0:c0 + WSZ], ps2[:],
                                        a1[:], c1[:], ALU.mult, ALU.add)
                for s in range(w * TPW, min((w + 1) * TPW, NT)):
                    pt = ppsum.tile([128, 128], bf16, tag="work")
                    nc.tensor.transpose(pt[:], xT[:, s * 128:(s + 1) * 128],
                                        ident_bf[:])
                    nc.scalar.copy(xnm[:, s * 128:(s + 1) * 128], pt[:])
                    B = pool_tile(s, ps_pos, pos_nm, C_DIM,
                                  s == 0, s == NT - 1)
                    nc.tensor.matmul(ps_x1, xnm[:, s * 128:(s + 1) * 128],
                                     B[:], start=(s == 0), stop=(s == NT - 1))

            c1_state = {"win": 0, "chunk": 0}

            def conv1_post_group(g):
                t_done = min((g + 1) * TGRP, NT)
                while (c1_state["win"] + 1) * TPW <= t_done:
                    conv1_win(c1_state["win"])
                    c1_state["win"] += 1
                while (c1_state["chunk"] < NCHUNK
                       and (c1_state["chunk"] + 1) * QT
                       <= c1_state["win"] * TPW):
                    cc_dma(c1_state["chunk"])
                    c1_state["chunk"] += 1

            seg_sweep(flush1, C_DIM, seg1_post, land1, conv1_post_group)
            assert c1_state["win"] == NW and c1_state["chunk"] == NCHUNK

            gstate = {}

            def ar1_block():
                arin_pos = pgr.tile([C_DIM, GWIN], bf16, tag="arin_p")
                nc.scalar.copy(arin_pos[:], ps_pos)
                arin_x1 = pgr.tile([H_DIM, GWIN], bf16, tag="arin", bufs=2)
                nc.scalar.copy(arin_x1[:], ps_x1)
                with nc.gpsimd.register("g0r") as g0r:
                    nc.gpsimd.reg_load(g0r, groff[0:1, 0:1])
                    sv = nc.gpsimd.snap(g0r, min_val=0, max_val=WG - GWIN)
                nc.gpsimd.dma_start(out=ar1_in[0:H_DIM, bass.ds(sv, GWIN)],
                                    in_=arin_x1[:])
                nc.gpsimd.dma_start(out=ar1_in[H_DIM:, bass.ds(sv, GWIN)],
                                    in_=arin_pos[:])
                nc.gpsimd.collective_compute(
                    "AllReduce", mybir.AluOpType.add,
                    ins=[ar1_in.opt()], outs=[ar1_out.opt()],
                    replica_groups=[list(range(NCORES))])

            def x01g_block():
                ar1x = pgr.tile([H_DIM, WG], bf16, tag="arbig", bufs=2)
                nc.sync.dma_start(out=ar1x[:], in_=ar1_out[0:H_DIM, :])
                ar1p = pgr.tile([C_DIM, WG], bf16, tag="ar1p")
                nc.sync.dma_start(out=ar1p[:], in_=ar1_out[H_DIM:, :])
                x0g = pgr.tile([H_DIM, WG], f32, tag="g_x0g")
                g_mlp(Wf1_bf, ar1p[:], x0g, V["bf1"], (af1, cf1))
                tmp = pgr.tile([H_DIM, WG], f32, tag="g_tmp")
                nc.vector.tensor_tensor(tmp[:], x0g[:], ar1x[:], op=ALU.add)
                x1g = pgr.tile([H_DIM, WG], f32, tag="g_x1g", bufs=2)
                g_mlp(W["Wf2"], tmp, x1g, V["bf2"], (af2, cf2))
                gstate.update(x0g=x0g, tmp=tmp, x1g=x1g)

            def g_mlp(lhsT_w, rhs, out, bias, bn, P=H_DIM, relu=True):
                for w in range(-(-WG // 512)):
                    c0 = w * 512
                    cw = min(512, WG - c0)
                    ps = ppsum.tile([P, 512], f32, tag="work")
                    nc.tensor.matmul(ps[:P, :cw], lhsT_w[:], rhs[:, c0:c0 + cw],
                                     start=True, stop=True)
                    fn = AF.Relu if relu else AF.Identity
                    nc.scalar.activation(ps[:P, :cw], ps[:P, :cw], fn,
                                         bias=bias[:], scale=1.0)
                    if bn is not None:
                        a_, c_ = bn
                        nc.vector.tensor_scalar(out[:, c0:c0 + cw], ps[:P, :cw],
                                                a_[:], c_[:], ALU.mult, ALU.add)
                    else:
                        nc.scalar.copy(out[:, c0:c0 + cw], ps[:P, :cw])

            Wf1_bf = pconst.tile([C_DIM, H_DIM], bf16, tag="Wf1_bf")
            nc.vector.tensor_copy(Wf1_bf[:], W["Wf1"][:])

            # ================= conv2 =================
            # seg sweep with interleaved MLP windows, x2 transposes and graph
            # pooling, mirroring conv1.
            ps_x2_t = ppool.tile([128, GWIN], f32, tag="acc")
            ps_x2 = ps_x2_t[:]

            def flush2(t, ps):
                cols = slice(t * 128, (t + 1) * 128)
                nc.vector.tensor_tensor(xT[:, cols], xT[:, cols], ps[:],
                                        op=ALU.add)

            def conv2_win(w):
                c0 = w * WSZ
                zwin = psmall.tile([128, WSZ], bf16, tag="zwin", bufs=2)
                ps = ppsum.tile([H_DIM, WSZ], f32, tag="work")
                nc.tensor.matmul(ps[:], W2a_bf[:], xT[:, c0:c0 + WSZ],
                                 start=True, stop=True)
                nc.scalar.activation(zwin[:], ps[:], AF.Relu,
                                     bias=V["b2a"][:], scale=1.0)
                ps2 = ppsum.tile([H_DIM, WSZ], f32, tag="work")
                nc.tensor.matmul(ps2[:], W2b_bf[:], zwin[:],
                                 start=True, stop=True)
                nc.scalar.activation(ps2[:], ps2[:], AF.Relu,
                                     bias=V["b2b"][:], scale=1.0)
                nc.vector.tensor_scalar(xT[:, c0:c0 + WSZ], ps2[:],
                                        a2[:], c2[:], ALU.mult, ALU.add)
                for s in range(w * TPW, min((w + 1) * TPW, NT)):
                    pt = ppsum.tile([128, 128], bf16, tag="work")
                    nc.tensor.transpose(pt[:], xT[:, s * 128:(s + 1) * 128],
                                        ident_bf[:])
                    nc.scalar.copy(xnm[:, s * 128:(s + 1) * 128], pt[:])
                    B = pool_tile(s, ps_x2, xnm, 128, s == 0, s == NT - 1)

            c2_state = {"win": 0}

            def conv2_post_group(g):
                t_done = min((g + 1) * TGRP, NT)
                while (c2_state["win"] + 1) * TPW <= t_done:
                    conv2_win(c2_state["win"])
                    c2_state["win"] += 1

            # chunk-outer conv2: gather units (c, g) with AllGather triggers
            # interleaved so the in-order gpsimd queue never waits on a
            # not-yet-ready collective. Per unit: 16 seg matmuls into per-tile
            # PSUM, then an SBUF accumulate into xT.
            units = [(c, g) for c in range(NCHUNK) for g in range(NGRP)]
            BPU = TGRP * SLC // 8        # S8 batches per unit
            s82 = {}
            lands2 = {}
            pump_state = {"issued": 0}
            AHEAD = 4

            def build_unit(i):
                c, g = units[i]
                for b in range(BPU):
                    s8i = (c * NGRP + g) * BPU + b
                    if s8i * 8 < NSLOT_T and s8i not in s82:
                        s82[s8i] = build_s8(dwc2, s8i * 8,
                                            min(8, NSLOT_T - s8i * 8))

            def gather2(j):
                c, g = units[j]
                base = ((c * NGRP + g) * TGRP) * CAPC
                land = pland.tile([128, CALL // 128, H_DIM], bf16, tag="land")
                nc.gpsimd.dma_gather(
                    land[:], cc_out[c][:],
                    gidx2[:, base // 16:(base + CALL) // 16],
                    CALL, CALL, H_DIM, single_packet=False,
                    queue_num=j % 4)
                return land

            def pump(upto):
                while pump_state["issued"] <= min(upto, len(units) - 1):
                    j = pump_state["issued"]
                    if j == NGRP:
                        cc_trig(2)
                    if j == 2 * NGRP:
                        cc_trig(3)
                        ar1_block()
                    lands2[j] = gather2(j)
                    pump_state["issued"] += 1

            cc_trig(0)
            cc_trig(1)
            build_unit(0)
            for i, (c, g) in enumerate(units):
                if i + 1 < len(units):
                    build_unit(i + 1)
                pump(i + AHEAD)
                if i == 2 * NGRP:
                    x01g_block()
                land = lands2.pop(i)
                for tl in range(TGRP):
                    t = g * TGRP + tl
                    if t >= NT:
                        break
                    ps = pseg.tile([H_DIM, 128], f32, tag="seg")
                    for sl in range(SLC):
                        slot = ((c * NGRP + g) * TGRP + tl) * SLC + sl
                        S = s82[slot // 8]
                        nc.tensor.matmul(
                            ps[:], land[:, tl * SLC + sl, :],
                            S[:, (slot % 8) * 128:(slot % 8 + 1) * 128],
                            start=(sl == 0), stop=(sl == SLC - 1))
                    flush2(t, ps)
                for b in range(BPU):
                    s82.pop((c * NGRP + g) * BPU + b, None)
                if c == NCHUNK - 1:
                    conv2_post_group(g)
            assert c2_state["win"] == NW

            arin2 = pgr.tile([H_DIM, GWIN], bf16, tag="arin", bufs=2)
            nc.scalar.copy(arin2[:], ps_x2)
            with nc.gpsimd.register("g0r2") as g0r2:
                nc.gpsimd.reg_load(g0r2, groff[0:1, 0:1])
                sv2 = nc.gpsimd.snap(g0r2, min_val=0, max_val=WG - GWIN)
            nc.gpsimd.dma_start(out=ar2_in[:, bass.ds(sv2, GWIN)], in_=arin2[:])
            nc.gpsimd.collective_compute(
                "AllReduce", mybir.AluOpType.add,
                ins=[ar2_in.opt()], outs=[ar2_out.opt()],
                replica_groups=[list(range(NCORES))])


            # ================= graph stage =================
            x0g, tmp, x1g = gstate["x0g"], gstate["tmp"], gstate["x1g"]
            nc.vector.tensor_tensor(tmp[:], x0g[:], x1g[:], op=ALU.add)
            ar2 = pgr.tile([H_DIM, WG], bf16, tag="arbig", bufs=2)
            nc.sync.dma_start(out=ar2[:], in_=ar2_out[:])
            nc.vector.tensor_tensor(tmp[:], tmp[:], ar2[:], op=ALU.add)
            x2g = pgr.tile([H_DIM, WG], f32, tag="g_x0g")
            g_mlp(W["Wf2"], tmp, x2g, V["bf2"], (af2, cf2))

            hcls = pgr.tile([HC_DIM, WG], f32, tag="g_tmp")
            g_mlp(W["Wc1"], x2g, hcls, V["bc1"], (acl, ccl), P=HC_DIM, relu=False)
            hneg = pgr.tile([HC_DIM, WG], f32, tag="g_x1g", bufs=2)
            nc.vector.tensor_scalar(hneg[:], hcls[:], V["a_prelu_v"][:], None,
                                    ALU.mult)
            nc.vector.tensor_tensor(hcls[:], hcls[:], hneg[:], op=ALU.max)
            outT = pgr.tile([O_DIM, WG], f32, tag="g_x1g", bufs=2)
            g_mlp(W["Wc2"], hcls, outT, V["bc2"], None, P=O_DIM, relu=False)

            ngt = -(-G // 128)
            onm = pgr.tile([128, ngt * O_DIM], f32, tag="onm")
            for j in range(ngt):
                pt = ppsum.tile([128, 128], f32, tag="work")
                nc.tensor.transpose(pt[:, 0:O_DIM], outT[:, j * 128:(j + 1) * 128],
                                    ident[0:O_DIM, 0:O_DIM])
                nc.scalar.copy(onm[:, j * O_DIM:(j + 1) * O_DIM], pt[:, 0:O_DIM])
            nfull = G // 128
            if nfull:
                nc.sync.dma_start(
                    out=out_d.ap()[0:nfull * 128, :].rearrange(
                        "(s p) o -> p s o", p=128),
                    in_=onm[:, :nfull * O_DIM].rearrange(
                        "p (s o) -> p s o", o=O_DIM))
            rem = G - nfull * 128
            if rem:
                nc.sync.dma_start(out=out_d.ap()[nfull * 128:G, :],
                                  in_=onm[0:rem, nfull * O_DIM:(nfull + 1) * O_DIM])

    nc.compile()
    return nc


def _build_in_maps(inputs, dims, arrays):
    import ml_dtypes
    f = lambda x: np.ascontiguousarray(np.asarray(x, np.float32))
    col = lambda x: f(x).reshape(-1, 1)
    shared = {
        "iota": np.tile(np.arange(max(dims["GWIN"], 128), dtype=np.float32),
                        (128, 1)).astype(ml_dtypes.bfloat16),
        "W1a": f(inputs["W1a"]), "W1b": f(inputs["W1b"]),
        "W2a": f(inputs["W2a"]), "W2b": f(inputs["W2b"]),
        "Wf1": f(inputs["Wf1"]), "Wf2": f(inputs["Wf2"]),
        "Wc1": f(inputs["Wc1"]), "Wc2": f(inputs["Wc2"]),
        "b1a": col(inputs["b1a"]), "b1b": col(inputs["b1b"]),
        "b2a": col(inputs["b2a"]), "b2b": col(inputs["b2b"]),
        "bf1": col(inputs["bf1"]), "bf2": col(inputs["bf2"]),
        "bc1": col(inputs["bc1"]), "bc2": col(inputs["bc2"]),
        "gc": col(inputs["gc"]), "bec": col(inputs["bec"]),
        "rmc": col(inputs["rmc"]), "rvc": col(inputs["rvc"]),
        "a_prelu_v": np.full((HC_DIM, 1),
                             np.float32(np.asarray(inputs["a_prelu"]))),
    }
    for pfx in ["n1_", "n2_", "f1_", "f2_"]:
        for sfx in ["g", "b", "rm", "rv"]:
            shared[pfx + sfx] = col(inputs[pfx + sfx])
    in_maps = []
    for k in range(NCORES):
        m = dict(shared)
        m["posE"] = arrays["posE"][k].astype(ml_dtypes.bfloat16)
        m["pos_nm"] = arrays["pos_nm"][k].astype(ml_dtypes.bfloat16)
        m["batch_rel"] = arrays["batch_rel"][k]
        m["gidx2"] = arrays["gidx2"][k]
        m["dwc"] = arrays["dwc"][k]
        m["dwc2"] = arrays["dwc2"][k]
        m["groff"] = arrays["groff"][k]
        in_maps.append(m)
    return in_maps


def _get_compiled(pos, edge_index, batch, N, E, G):
    dims, arrays = _preprocess(pos, edge_index, batch, N, E, G)
    key = tuple(sorted((k, v) for k, v in dims.items()))
    if key not in _CACHE:
        _CACHE[key] = _build_program(dims)
    return _CACHE[key], dims, arrays


def kernel(**inputs):
    from concourse.bass_utils import run_bass_kernel_spmd
    pos = np.asarray(inputs["pos"])
    ei = np.asarray(inputs["edge_index"])
    batch = np.asarray(inputs["batch"])
    nc, dims, arrays = _get_compiled(pos, ei, batch, pos.shape[0],
                                     ei.shape[1], G_FULL)
    in_maps = _build_in_maps(inputs, dims, arrays)
    res = run_bass_kernel_spmd(nc, in_maps, list(range(NCORES)))
    return np.asarray(res.results[0]["out"], np.float32)



# revision 54
# speedup vs baseline: 1.3269x; 1.3269x over previous
"""Trainium2 Bass kernel for the DSCNMP GNN (2x GINConv + pooling + MLP head).

Self-contained: takes full (unsharded) inputs, shards nodes/edges across the
8 NeuronCores internally, runs one SPMD Bass program via
bass_utils.run_bass_kernel_spmd, and returns the full [G, O] output.

Sharding strategy (per the problem's hint):
  - Nodes partitioned contiguously across 8 cores; each edge owned by the
    core of its dst node. Small MLP/BN weights replicated.
  - Per-edge gathers via dma_gather from padded node tables in DRAM
    (4 chunks to satisfy the int16 index range).
  - Segment-sum on the TensorEngine: per 128-edge slot, a one-hot selection
    matrix S (S[e, j] = dst_e == j) is built on DVE and agg^T accumulates in
    PSUM as land^T @ S per 128-node tile (dma_scatter_add loses duplicate
    updates on HW, so scatter is not used).
  - x1 node features AllGathered (bf16) so every core can gather remote rows.
  - Pooled graph embeddings AllReduced; graph-level head replicated.

All floating-point math happens on device; the host does integer index and
layout preprocessing only.
"""

import numpy as np

N_FULL, E_FULL, G_FULL, C_DIM, H_DIM, O_DIM = 100000, 600000, 1000, 2, 128, 10
HC_DIM = H_DIM // 2
NCORES = 8
NCHUNK = 4          # int16 gather-index chunking of the global table
TGRP = 8            # node tiles per gather group
EPS = 1e-5

_CACHE = {}


def _pack_idx16(flat):
    """[j%16, j//16] int16 packing, replicated across the 8 Q7 groups."""
    total = len(flat)
    assert total % 16 == 0
    out = flat.reshape(total // 16, 16).T.astype(np.int16)
    return np.tile(out, (8, 1))


def _preprocess(pos, edge_index, batch, N, E, G):
    NL = N // NCORES
    NLP = -(-NL // 512) * 512
    NT = NLP // 128
    NGRP = -(-NT // TGRP)
    NTP = NGRP * TGRP
    TOT = NLP * NCORES
    QL = NLP // NCHUNK
    CHUNK = QL * NCORES          # rows per quarter-table
    assert CHUNK <= 32767 and QL % 128 == 0

    pos = np.asarray(pos, np.float32)
    src = np.asarray(edge_index[0], np.int64)
    dst = np.asarray(edge_index[1], np.int64)
    batch = np.asarray(batch, np.int64)
    assert N % NCORES == 0

    node = np.arange(N)
    slot_of = NLP * (node // NL) + (node % NL)

    posT = np.zeros((NCORES, C_DIM, NLP), np.float32)   # only for in_maps ref
    pos_nm = np.zeros((NCORES, 128, NT * C_DIM), np.float32)
    batch_rel = np.full((NCORES, 128, NT), -5.0, np.float32)
    g0 = np.zeros(NCORES, np.int64)
    gwin_need = 0
    for k in range(NCORES):
        nodes = np.arange(k * NL, (k + 1) * NL)
        j = nodes - k * NL
        posT[k][:, j] = pos[nodes].T
        pos_nm[k][j % 128, (j // 128) * C_DIM + 0] = pos[nodes, 0]
        pos_nm[k][j % 128, (j // 128) * C_DIM + 1] = pos[nodes, 1]
        g0[k] = batch[nodes[0]]
        rel = batch[nodes] - g0[k]
        batch_rel[k][j % 128, j // 128] = rel.astype(np.float32)
        gwin_need = max(gwin_need, int(rel.max()) + 1)
    GWIN = min(512, max(128, -(-gwin_need // 32) * 32))
    assert gwin_need <= GWIN <= 512
    WG = -(-(G + GWIN) // 256) * 256


    # ---- edge cells: (core, quarter-table, node-tile) ----
    ecore = dst // NL
    ksrc = slot_of[src] // NLP
    jsrc = slot_of[src] % NLP
    grow_q = ksrc * QL + (jsrc % QL)     # row within quarter-table
    qsrc = jsrc // QL
    dslot = slot_of[dst] % NLP
    cells = [[None] * (NCHUNK * NT) for _ in range(NCORES)]
    capc = 128
    for k in range(NCORES):
        m = ecore == k
        gs, ds_ = grow_q[m], dslot[m]
        ch = qsrc[m]
        tt = ds_ // 128
        for c in range(NCHUNK):
            for t in range(NT):
                mm = (ch == c) & (tt == t)
                gg, dd = gs[mm], ds_[mm]
                o = np.argsort(dd, kind="stable")
                cells[k][c * NT + t] = (gg[o], dd[o] - t * 128)
                capc = max(capc, -(-len(gg) // 128) * 128)
    CAPC = capc
    SLC = CAPC // 128                     # slots per cell
    STREAM = NGRP * NCHUNK * TGRP * CAPC  # total padded positions
    NSLOT = STREAM // 128

    gidx2 = np.zeros((NCORES, 128, STREAM // 16), np.int16)
    dwc = np.full((NCORES, 128, NSLOT), -5.0, np.float32)
    dwc2 = np.full((NCORES, 128, NSLOT), -5.0, np.float32)
    posE = np.zeros((NCORES, 128, NSLOT * C_DIM), np.float32)
    invq = np.full((NCHUNK, CHUNK), -1, np.int64)
    invq[qsrc * 0 + 0, 0] = 0  # placeholder shape init
    invq = np.full((NCHUNK, CHUNK), 0, np.int64)
    allk = slot_of // NLP
    allj = slot_of % NLP
    invq[allj // QL, allk * QL + (allj % QL)] = np.arange(N)
    for k in range(NCORES):
        gi2 = np.zeros(STREAM, np.int64)
        dw = np.full(STREAM, -5.0, np.float32)
        dw2 = np.full(STREAM, -5.0, np.float32)
        pe = np.zeros((STREAM, C_DIM), np.float32)
        for g in range(NGRP):
            for c in range(NCHUNK):
                for tl in range(TGRP):
                    t = g * TGRP + tl
                    base = ((g * NCHUNK + c) * TGRP + tl) * CAPC
                    base2 = ((c * NGRP + g) * TGRP + tl) * CAPC
                    if t < NT:
                        gg, dd = cells[k][c * NT + t]
                        gi2[base2:base2 + len(gg)] = gg
                        dw[base:base + len(dd)] = dd.astype(np.float32)
                        dw2[base2:base2 + len(dd)] = dd.astype(np.float32)
                        pe[base:base + len(gg)] = pos[invq[c, gg]]
        gidx2[k] = _pack_idx16(gi2)
        dwc[k] = dw.reshape(NSLOT, 128).T
        dwc2[k] = dw2.reshape(NSLOT, 128).T
        posE[k] = pe.reshape(NSLOT, 128, C_DIM).transpose(1, 0, 2).reshape(
            128, NSLOT * C_DIM)
    groff = np.zeros((NCORES, 1, 2), np.int32)
    groff[:, 0, 0] = g0
    assert (g0 + GWIN <= WG).all()

    dims = dict(N=N, E=E, G=G, NL=NL, NLP=NLP, NT=NT, NGRP=NGRP, NTP=NTP,
                TOT=TOT, CHUNK=CHUNK, CAPC=CAPC, SLC=SLC, STREAM=STREAM,
                GWIN=GWIN, WG=WG)
    arrays = dict(posE=posE, pos_nm=pos_nm, batch_rel=batch_rel,
                  gidx2=gidx2, dwc=dwc, dwc2=dwc2, groff=groff)
    return dims, arrays


def _build_program(dims):
    import contextlib
    import concourse.bass as bass
    import concourse.bacc as bacc
    import concourse.mybir as mybir
    import concourse.tile as tile
    from concourse import library_config
    from concourse.masks import make_identity

    f32 = mybir.dt.float32
    bf16 = mybir.dt.bfloat16
    i16 = mybir.dt.int16
    i32 = mybir.dt.int32
    AF = mybir.ActivationFunctionType
    ALU = mybir.AluOpType

    NLP, NT, NGRP = dims["NLP"], dims["NT"], dims["NGRP"]
    QL = NLP // NCHUNK
    TOT, CHUNK, CAPC, SLC = dims["TOT"], dims["CHUNK"], dims["CAPC"], dims["SLC"]
    STREAM, GWIN, WG, G = dims["STREAM"], dims["GWIN"], dims["WG"], dims["G"]
    CALL = TGRP * CAPC              # idxs per dma_gather call
    WSZ = next(w for w in (512, 384, 256, 128) if NLP % w == 0)
    NW = NLP // WSZ                 # windows for MLP sweeps

    nc = bacc.Bacc("TRN2", target_bir_lowering=False, debug=False,
                   enable_asserts=True, num_devices=NCORES,
                   num_swdge_queues=4)

    def din(name, shape, dt=f32):
        return nc.dram_tensor(name, list(shape), dt, kind="ExternalInput")

    posE_d = din("posE", [128, (STREAM // 128) * C_DIM], bf16)
    pos_nm_d = din("pos_nm", [128, NT * C_DIM], bf16)
    batch_rel_d = din("batch_rel", [128, NT])
    gidx2_d = din("gidx2", [128, STREAM // 16], i16)
    dwc_d = din("dwc", [128, STREAM // 128])
    dwc2_d = din("dwc2", [128, STREAM // 128])
    groff_d = din("groff", [1, 2], i32)
    iota_d = din("iota", [128, max(GWIN, 128)], bf16)

    wnames = {}
    for nm, shp in [("W1a", [C_DIM, H_DIM]), ("W1b", [H_DIM, H_DIM]),
                    ("W2a", [H_DIM, H_DIM]), ("W2b", [H_DIM, H_DIM]),
                    ("Wf1", [C_DIM, H_DIM]), ("Wf2", [H_DIM, H_DIM]),
                    ("Wc1", [H_DIM, HC_DIM]), ("Wc2", [HC_DIM, O_DIM])]:
        wnames[nm] = din(nm, shp)
    vecs = {}
    for nm in ["b1a", "b1b", "b2a", "b2b", "bf1", "bf2",
               "n1_g", "n1_b", "n1_rm", "n1_rv", "n2_g", "n2_b", "n2_rm", "n2_rv",
               "f1_g", "f1_b", "f1_rm", "f1_rv", "f2_g", "f2_b", "f2_rm", "f2_rv"]:
        vecs[nm] = din(nm, [H_DIM, 1])
    for nm in ["bc1", "gc", "bec", "rmc", "rvc", "a_prelu_v"]:
        vecs[nm] = din(nm, [HC_DIM, 1])
    vecs["bc2"] = din("bc2", [O_DIM, 1])

    out_d = nc.dram_tensor("out", [G, O_DIM], f32, kind="ExternalOutput")

    with tile.TileContext(nc) as tc:
        nc.gpsimd.load_library(library_config.mlp)
        ctx = contextlib.ExitStack()
        with ctx:
            dram = ctx.enter_context(tc.tile_pool(name="dram", bufs=1, space="DRAM"))
            pconst = ctx.enter_context(tc.tile_pool(name="const", bufs=1))
            pbig = ctx.enter_context(tc.tile_pool(name="big", bufs=1))
            pland = ctx.enter_context(tc.tile_pool(name="land", bufs=8))
            psmall = ctx.enter_context(tc.tile_pool(name="small", bufs=4))
            pgr = ctx.enter_context(tc.tile_pool(name="gr", bufs=1))
            ph1 = ctx.enter_context(tc.tile_pool(name="h1w", bufs=2))
            ppsum = ctx.enter_context(tc.tile_pool(name="psum", bufs=2, space="PSUM"))
            pseg = ctx.enter_context(tc.tile_pool(name="psum_seg", bufs=4, space="PSUM"))
            ppool = ctx.enter_context(tc.tile_pool(name="psum_acc", bufs=2, space="PSUM"))

            cc_in = [dram.tile([QL, H_DIM], bf16, tag="cc_in", name=f"cc_in{q}",
                                bufs=NCHUNK) for q in range(NCHUNK)]
            cc_out = [dram.tile([QL * NCORES, H_DIM], bf16, tag="cc_out",
                                name=f"cc_out{q}", addr_space="Shared",
                                bufs=NCHUNK) for q in range(NCHUNK)]
            ar1_in = dram.tile([H_DIM + C_DIM, WG], bf16, tag="ar1_in")
            ar1_out = dram.tile([H_DIM + C_DIM, WG], bf16, tag="ar1_out", addr_space="Shared")
            ar2_in = dram.tile([H_DIM, WG], bf16, tag="ar2_in")
            ar2_out = dram.tile([H_DIM, WG], bf16, tag="ar2_out", addr_space="Shared")

            _ld_engines = [nc.sync, nc.scalar]
            _ld_state = {"i": 0}

            def load_const(dr, shape, dt=f32):
                t = pconst.tile(shape, dt, tag=dr.name + "_sb")
                eng = _ld_engines[_ld_state["i"] % len(_ld_engines)]
                _ld_state["i"] += 1
                eng.dma_start(out=t[:], in_=dr.ap())
                return t

            W = {k: load_const(v, v.shape) for k, v in wnames.items()}
            V = {k: load_const(v, v.shape) for k, v in vecs.items()}
            pos_nm = load_const(pos_nm_d, [128, NT * C_DIM], bf16)
            posE = load_const(posE_d, [128, (STREAM // 128) * C_DIM], bf16)
            batch_rel = load_const(batch_rel_d, [128, NT])
            iota_bf = load_const(iota_d, [128, max(GWIN, 128)], bf16)
            gidx2 = load_const(gidx2_d, [128, STREAM // 16], i16)
            dwc = load_const(dwc_d, [128, STREAM // 128])
            dwc2 = load_const(dwc2_d, [128, STREAM // 128])
            groff = load_const(groff_d, [1, 2], i32)

            ident = pconst.tile([128, 128], f32, tag="ident")
            make_identity(nc, ident[:])

            def bn_vec(g, b, rm, rv, P, nm):
                a = pconst.tile([P, 1], f32, tag=f"bn_a_{nm}")
                c = pconst.tile([P, 1], f32, tag=f"bn_c_{nm}")
                nc.vector.tensor_scalar(a[:], rv[:], EPS, None, ALU.add)
                nc.scalar.activation(a[:], a[:], AF.Sqrt)
                nc.vector.reciprocal(a[:], a[:])
                nc.vector.tensor_tensor(a[:], a[:], g[:], op=ALU.mult)
                nc.vector.tensor_tensor(c[:], rm[:], a[:], op=ALU.mult)
                nc.vector.tensor_tensor(c[:], b[:], c[:], op=ALU.subtract)
                return a, c
            a1, c1 = bn_vec(V["n1_g"], V["n1_b"], V["n1_rm"], V["n1_rv"], H_DIM, "n1")
            a2, c2 = bn_vec(V["n2_g"], V["n2_b"], V["n2_rm"], V["n2_rv"], H_DIM, "n2")
            af1, cf1 = bn_vec(V["f1_g"], V["f1_b"], V["f1_rm"], V["f1_rv"], H_DIM, "f1")
            af2, cf2 = bn_vec(V["f2_g"], V["f2_b"], V["f2_rm"], V["f2_rv"], H_DIM, "f2")
            acl, ccl = bn_vec(V["gc"], V["bec"], V["rmc"], V["rvc"], HC_DIM, "cls")

            # persistent big buffers
            xT = pbig.tile([128, NLP], bf16, tag="B")        # x1T -> h2T -> (x2T)
            xnm = pbig.tile([128, NT * H_DIM], bf16, tag="NM")  # x1nm then x2nm

            # ---------------- segment-sum sweep (shared structure) ----------
            # S matrices for 8 consecutive 128-edge slots are built in ONE
            # DVE op: S8[p, s*128+j] = (iota128[j] == dwc[p, s0+s]) via
            # step-0 broadcast APs on both operands. S8s only depend on the
            # static dwc table, so they are prebuilt one group ahead (bufs=12
            # on the S8 tag) to keep the PE fed.
            NSLOT_T = STREAM // 128
            NS8 = -(-NSLOT_T // 8)

            def build_s8(dwc_t, s0, nsl):
                S8 = psmall.tile([128, 8 * 128], bf16, tag="S8", bufs=12)
                nc.vector.tensor_tensor(
                    out=S8[:, 0:nsl * 128].rearrange("p (s j) -> p s j", j=128),
                    in0=iota_bf[:, None, 0:128].to_broadcast([128, nsl, 128]),
                    in1=dwc_t[:, s0:s0 + nsl, None].to_broadcast(
                        [128, nsl, 128]),
                    op=ALU.is_equal)
                return S8

            def seg_sweep(flush_fn, lhs_cols, post_mm, land_fn, post_group):
                s8tiles = {}

                def build8(g):
                    for k in range(8):
                        s8i = g * 8 + k
                        if s8i >= NS8:
                            return
                        ns8 = min(8, NSLOT_T - s8i * 8)
                        s8tiles[s8i] = build_s8(dwc, s8i * 8, ns8)

                build8(0)
                for g in range(NGRP):
                    if g + 1 < NGRP:
                        build8(g + 1)
                    lands = [land_fn(g, c) for c in range(NCHUNK)]
                    for tl in range(TGRP):
                        t = g * TGRP + tl
                        if t >= NT:
                            break
                        ps = pseg.tile([lhs_cols, 128], f32, tag="seg")
                        first = True
                        for c in range(NCHUNK):
                            for sl in range(SLC):
                                slot = ((g * NCHUNK + c) * TGRP + tl) * SLC + sl
                                S = s8tiles[slot // 8]
                                soff = (slot % 8) * 128
                                nc.tensor.matmul(
                                    ps[:], lands[c](tl, sl),
                                    S[:, soff:soff + 128], start=first,
                                    stop=False)
                                first = False
                        post_mm(t, ps)
                        flush_fn(t, ps)
                    for k in range(8):
                        s8tiles.pop(g * 8 + k, None)
                    post_group(g)

            # ================= conv1 =================
            # psum[t] rows 0:2 = agg(pos)^T tile; pos^T added via matmul vs
            # identity (lhsT dtype must match rhs -> bf16 identity).
            ident_bf = pconst.tile([128, 128], bf16, tag="ident_bf")
            nc.vector.tensor_copy(ident_bf[:], ident[:])
            W1b_bf = pconst.tile([H_DIM, H_DIM], bf16, tag="W1b_bf")
            nc.vector.tensor_copy(W1b_bf[:], W["W1b"][:])
            W2a_bf = pconst.tile([H_DIM, H_DIM], bf16, tag="W2a_bf")
            nc.vector.tensor_copy(W2a_bf[:], W["W2a"][:])
            W2b_bf = pconst.tile([H_DIM, H_DIM], bf16, tag="W2b_bf")
            nc.vector.tensor_copy(W2b_bf[:], W["W2b"][:])
            W1a_bf = pconst.tile([C_DIM, H_DIM], bf16, tag="W1a_bf")
            nc.vector.tensor_copy(W1a_bf[:], W["W1a"][:])
            h1w = {}

            def seg1_post(t, ps):
                nc.tensor.matmul(ps[:], pos_nm[:, t * C_DIM:(t + 1) * C_DIM],
                                 ident_bf[:], start=False, stop=True)

            def flush1(t, ps):
                w, sub = divmod(t * 128, WSZ)
                if w not in h1w:
                    h1w[w] = ph1.tile([C_DIM, WSZ], bf16, tag="h1w",
                                      name=f"h1w{w}", bufs=4)
                nc.scalar.copy(h1w[w][:, sub:sub + 128], ps[0:C_DIM, :])

            posE_v = posE[:].rearrange("p (s c) -> p s c", c=C_DIM)

            def land1(g, c):
                def get(tl, sl):
                    slot = ((g * NCHUNK + c) * TGRP + tl) * SLC + sl
                    return posE_v[:, slot, :]
                return get

            # conv1 MLP + transpose for one window (4 node tiles), then
            # AllGather a chunk of the x1 table as soon as it is complete —
            # the 4 AllGathers hide under the rest of the conv1 sweep.
            QT = QL // 128
            TPW = WSZ // 128  # tiles per MLP window

            def cc_dma(q):
                nc.sync.dma_start(
                    out=cc_in[q][:].rearrange("(s p) f -> p s f", p=128),
                    in_=xnm[:, q * QT * H_DIM:(q + 1) * QT * H_DIM].rearrange(
                        "p (s f) -> p s f", f=H_DIM))

            def cc_trig(q):
                nc.gpsimd.collective_compute(
                    "AllGather", mybir.AluOpType.bypass,
                    ins=[cc_in[q].opt()], outs=[cc_out[q].opt()],
                    replica_groups=[list(range(NCORES))])

            # zero-fill the AllReduce scratch windows up front
            zrow = pgr.tile([H_DIM, 256], bf16, tag="zrow")
            nc.vector.memset(zrow[:], 0.0)
            for zc in range(0, WG, 256):
                nc.sync.dma_start(out=ar1_in[0:H_DIM, zc:zc + 256], in_=zrow[:])
                nc.sync.dma_start(out=ar1_in[H_DIM:, zc:zc + 256],
                                  in_=zrow[0:C_DIM, :])
                nc.sync.dma_start(out=ar2_in[:, zc:zc + 256], in_=zrow[:])

            ps_pos_t = ppool.tile([C_DIM, GWIN], f32, tag="acc")
            ps_x1_t = ppool.tile([128, GWIN], f32, tag="acc")
            ps_pos = ps_pos_t[:]
            ps_x1 = ps_x1_t[:]

            def pool_tile(s, psacc, table, cols, first, last):
                B = psmall.tile([128, GWIN], bf16, tag="B")
                nc.vector.tensor_scalar(B[:], iota_bf[:, 0:GWIN],
                                        batch_rel[:, s:s + 1], None,
                                        ALU.is_equal)
                nc.tensor.matmul(psacc, table[:, s * cols:(s + 1) * cols],
                                 B[:], start=first, stop=last)
                return B

            def conv1_win(w):
                c0 = w * WSZ
                zwin = psmall.tile([128, WSZ], bf16, tag="zwin", bufs=2)
                ps = ppsum.tile([H_DIM, WSZ], f32, tag="work")
                nc.tensor.matmul(ps[:], W1a_bf[:], h1w[w][:],
                                 start=True, stop=True)
                nc.scalar.activation(zwin[:], ps[:], AF.Relu,
                                     bias=V["b1a"][:], scale=1.0)
                ps2 = ppsum.tile([H_DIM, WSZ], f32, tag="work")
                nc.tensor.matmul(ps2[:], W1b_bf[:], zwin[:],
                                 start=True, stop=True)
                nc.scalar.activation(ps2[:], ps2[:], AF.Relu,
                                     bias=V["b1b"][:], scale=1.0)
                nc.vector.tensor_scalar(xT[:, c0:c0 + WSZ], ps2[:],
                                        a1[:], c1[:], ALU.mult, ALU.add)
                for s in range(w * TPW, min((w + 1) * TPW, NT)):
                    pt = ppsum.tile([128, 128], bf16, tag="work")
                    nc.tensor.transpose(pt[:], xT[:, s * 128:(s + 1) * 128],
                                        ident_bf[:])
                    nc.scalar.copy(xnm[:, s * 128:(s + 1) * 128], pt[:])
                    B = pool_tile(s, ps_pos, pos_nm, C_DIM,
                                  s == 0, s == NT - 1)
                    nc.tensor.matmul(ps_x1, xnm[:, s * 128:(s + 1) * 128],
                                     B[:], start=(s == 0), stop=(s == NT - 1))

            c1_state = {"win": 0, "chunk": 0}

            def conv1_post_group(g):
                t_done = min((g + 1) * TGRP, NT)
                while (c1_state["win"] + 1) * TPW <= t_done:
                    conv1_win(c1_state["win"])
                    c1_state["win"] += 1
                while (c1_state["chunk"] < NCHUNK
                       and (c1_state["chunk"] + 1) * QT
                       <= c1_state["win"] * TPW):
                    cc_dma(c1_state["chunk"])
                    cc_trig(c1_state["chunk"])
                    c1_state["chunk"] += 1

            seg_sweep(flush1, C_DIM, seg1_post, land1, conv1_post_group)
            assert c1_state["win"] == NW and c1_state["chunk"] == NCHUNK

            gstate = {}

            def ar1_block():
                arin_pos = pgr.tile([C_DIM, GWIN], bf16, tag="arin_p")
                nc.scalar.copy(arin_pos[:], ps_pos)
                arin_x1 = pgr.tile([H_DIM, GWIN], bf16, tag="arin", bufs=2)
                nc.scalar.copy(arin_x1[:], ps_x1)
                with nc.gpsimd.register("g0r") as g0r:
                    nc.gpsimd.reg_load(g0r, groff[0:1, 0:1])
                    sv = nc.gpsimd.snap(g0r, min_val=0, max_val=WG - GWIN)
                nc.gpsimd.dma_start(out=ar1_in[0:H_DIM, bass.ds(sv, GWIN)],
                                    in_=arin_x1[:])
                nc.gpsimd.dma_start(out=ar1_in[H_DIM:, bass.ds(sv, GWIN)],
                                    in_=arin_pos[:])
                nc.gpsimd.collective_compute(
                    "AllReduce", mybir.AluOpType.add,
                    ins=[ar1_in.opt()], outs=[ar1_out.opt()],
                    replica_groups=[list(range(NCORES))])

            def x01g_block():
                ar1x = pgr.tile([H_DIM, WG], bf16, tag="arbig", bufs=2)
                nc.sync.dma_start(out=ar1x[:], in_=ar1_out[0:H_DIM, :])
                ar1p = pgr.tile([C_DIM, WG], bf16, tag="ar1p")
                nc.sync.dma_start(out=ar1p[:], in_=ar1_out[H_DIM:, :])
                x0g = pgr.tile([H_DIM, WG], f32, tag="g_x0g")
                g_mlp(Wf1_bf, ar1p[:], x0g, V["bf1"], (af1, cf1))
                tmp = pgr.tile([H_DIM, WG], f32, tag="g_tmp")
                nc.vector.tensor_tensor(tmp[:], x0g[:], ar1x[:], op=ALU.add)
                x1g = pgr.tile([H_DIM, WG], f32, tag="g_x1g", bufs=2)
                g_mlp(W["Wf2"], tmp, x1g, V["bf2"], (af2, cf2))
                gstate.update(x0g=x0g, tmp=tmp, x1g=x1g)

            def g_mlp(lhsT_w, rhs, out, bias, bn, P=H_DIM, relu=True):
                for w in range(-(-WG // 512)):
                    c0 = w * 512
                    cw = min(512, WG - c0)
                    ps = ppsum.tile([P, 512], f32, tag="work")
                    nc.tensor.matmul(ps[:P, :cw], lhsT_w[:], rhs[:, c0:c0 + cw],
                                     start=True, stop=True)
                    fn = AF.Relu if relu else AF.Identity
                    nc.scalar.activation(ps[:P, :cw], ps[:P, :cw], fn,
                                         bias=bias[:], scale=1.0)
                    if bn is not None:
                        a_, c_ = bn
                        nc.vector.tensor_scalar(out[:, c0:c0 + cw], ps[:P, :cw],
                                                a_[:], c_[:], ALU.mult, ALU.add)
                    else:
                        nc.scalar.copy(out[:, c0:c0 + cw], ps[:P, :cw])

            Wf1_bf = pconst.tile([C_DIM, H_DIM], bf16, tag="Wf1_bf")
            nc.vector.tensor_copy(Wf1_bf[:], W["Wf1"][:])

            # ================= conv2 =================
            # seg sweep with interleaved MLP windows, x2 transposes and graph
            # pooling, mirroring conv1.
            ps_x2_t = ppool.tile([128, GWIN], f32, tag="acc")
            ps_x2 = ps_x2_t[:]

            def flush2(t, ps):
                cols = slice(t * 128, (t + 1) * 128)
                nc.vector.tensor_tensor(xT[:, cols], xT[:, cols], ps[:],
                                        op=ALU.add)

            def conv2_win(w):
                c0 = w * WSZ
                zwin = psmall.tile([128, WSZ], bf16, tag="zwin", bufs=2)
                ps = ppsum.tile([H_DIM, WSZ], f32, tag="work")
                nc.tensor.matmul(ps[:], W2a_bf[:], xT[:, c0:c0 + WSZ],
                                 start=True, stop=True)
                nc.scalar.activation(zwin[:], ps[:], AF.Relu,
                                     bias=V["b2a"][:], scale=1.0)
                ps2 = ppsum.tile([H_DIM, WSZ], f32, tag="work")
                nc.tensor.matmul(ps2[:], W2b_bf[:], zwin[:],
                                 start=True, stop=True)
                nc.scalar.activation(ps2[:], ps2[:], AF.Relu,
                                     bias=V["b2b"][:], scale=1.0)
                nc.vector.tensor_scalar(xT[:, c0:c0 + WSZ], ps2[:],
                                        a2[:], c2[:], ALU.mult, ALU.add)
                for s in range(w * TPW, min((w + 1) * TPW, NT)):
                    pt = ppsum.tile([128, 128], bf16, tag="work")
                    nc.tensor.transpose(pt[:], xT[:, s * 128:(s + 1) * 128],
                                        ident_bf[:])
                    nc.scalar.copy(xnm[:, s * 128:(s + 1) * 128], pt[:])
                    B = pool_tile(s, ps_x2, xnm, 128, s == 0, s == NT - 1)

            c2_state = {"win": 0}

            def conv2_post_group(g):
                t_done = min((g + 1) * TGRP, NT)
                while (c2_state["win"] + 1) * TPW <= t_done:
                    conv2_win(c2_state["win"])
                    c2_state["win"] += 1

            # group-major conv2: per group, gather the 4 chunk units, then
            # per-tile 8-matmul PSUM chains across chunks, one xT add per
            # tile. Gathers are pumped AHEAD units forward.
            ar1_block()
            x01g_block()
            units = [(c, g) for g in range(NGRP) for c in range(NCHUNK)]
            BPU = TGRP * SLC // 8        # S8 batches per unit
            s82 = {}
            lands2 = {}
            pump_state = {"issued": 0}
            AHEAD = 5

            def build_unit(i):
                c, g = units[i]
                for b in range(BPU):
                    s8i = (c * NGRP + g) * BPU + b
                    if s8i * 8 < NSLOT_T and s8i not in s82:
                        s82[s8i] = build_s8(dwc2, s8i * 8,
                                            min(8, NSLOT_T - s8i * 8))

            def gather2(j):
                c, g = units[j]
                base = ((c * NGRP + g) * TGRP) * CAPC
                land = pland.tile([128, CALL // 128, H_DIM], bf16, tag="land")
                nc.gpsimd.dma_gather(
                    land[:], cc_out[c][:],
                    gidx2[:, base // 16:(base + CALL) // 16],
                    CALL, CALL, H_DIM, single_packet=False,
                    queue_num=j % 4)
                return land

            def pump(upto):
                while pump_state["issued"] <= min(upto, len(units) - 1):
                    j = pump_state["issued"]
                    lands2[j] = gather2(j)
                    pump_state["issued"] += 1

            for i in range(NCHUNK):
                build_unit(i)
            for g in range(NGRP):
                for i in range(NCHUNK * (g + 1), NCHUNK * (g + 2)):
                    if i < len(units):
                        build_unit(i)
                pump(NCHUNK * g + NCHUNK - 1 + AHEAD)
                lands = [lands2.pop(NCHUNK * g + c) for c in range(NCHUNK)]
                for tl in range(TGRP):
                    t = g * TGRP + tl
                    if t >= NT:
                        break
                    ps = pseg.tile([H_DIM, 128], f32, tag="seg")
                    first = True
                    for c in range(NCHUNK):
                        for sl in range(SLC):
                            slot = ((c * NGRP + g) * TGRP + tl) * SLC + sl
                            S = s82[slot // 8]
                            nc.tensor.matmul(
                                ps[:], lands[c][:, tl * SLC + sl, :],
                                S[:, (slot % 8) * 128:(slot % 8 + 1) * 128],
                                start=first,
                                stop=(c == NCHUNK - 1 and sl == SLC - 1))
                            first = False
                    flush2(t, ps)
                for c in range(NCHUNK):
                    for b in range(BPU):
                        s82.pop((c * NGRP + g) * BPU + b, None)
                conv2_post_group(g)
            assert c2_state["win"] == NW

            arin2 = pgr.tile([H_DIM, GWIN], bf16, tag="arin", bufs=2)
            nc.scalar.copy(arin2[:], ps_x2)
            with nc.gpsimd.register("g0r2") as g0r2:
                nc.gpsimd.reg_load(g0r2, groff[0:1, 0:1])
                sv2 = nc.gpsimd.snap(g0r2, min_val=0, max_val=WG - GWIN)
            nc.gpsimd.dma_start(out=ar2_in[:, bass.ds(sv2, GWIN)], in_=arin2[:])
            nc.gpsimd.collective_compute(
                "AllReduce", mybir.AluOpType.add,
                ins=[ar2_in.opt()], outs=[ar2_out.opt()],
                replica_groups=[list(range(NCORES))])


            # ================= graph stage =================
            x0g, tmp, x1g = gstate["x0g"], gstate["tmp"], gstate["x1g"]
            nc.vector.tensor_tensor(tmp[:], x0g[:], x1g[:], op=ALU.add)
            ar2 = pgr.tile([H_DIM, WG], bf16, tag="arbig", bufs=2)
            nc.sync.dma_start(out=ar2[:], in_=ar2_out[:])
            nc.vector.tensor_tensor(tmp[:], tmp[:], ar2[:], op=ALU.add)
            x2g = pgr.tile([H_DIM, WG], f32, tag="g_x0g")
            g_mlp(W["Wf2"], tmp, x2g, V["bf2"], (af2, cf2))

            hcls = pgr.tile([HC_DIM, WG], f32, tag="g_tmp")
            g_mlp(W["Wc1"], x2g, hcls, V["bc1"], (acl, ccl), P=HC_DIM, relu=False)
            hneg = pgr.tile([HC_DIM, WG], f32, tag="g_x1g", bufs=2)
            nc.vector.tensor_scalar(hneg[:], hcls[:], V["a_prelu_v"][:], None,
                                    ALU.mult)
            nc.vector.tensor_tensor(hcls[:], hcls[:], hneg[:], op=ALU.max)
            outT = pgr.tile([O_DIM, WG], f32, tag="g_x1g", bufs=2)
            g_mlp(W["Wc2"], hcls, outT, V["bc2"], None, P=O_DIM, relu=False)

            ngt = -(-G // 128)
            onm = pgr.tile([128, ngt * O_DIM], f32, tag="onm")
            for j in range(ngt):
                pt = ppsum.tile([128, 128], f32, tag="work")
                nc.tensor.transpose(pt[:, 0:O_DIM], outT[:, j * 128:(j + 1) * 128],
                                    ident[0:O_DIM, 0:O_DIM])
                nc.scalar.copy(onm[:, j * O_DIM:(j + 1) * O_DIM], pt[:, 0:O_DIM])
            nfull = G // 128
            if nfull:
                nc.sync.dma_start(
                    out=out_d.ap()[0:nfull * 128, :].rearrange(
                        "(s p) o -> p s o", p=128),
                    in_=onm[:, :nfull * O_DIM].rearrange(
                        "p (s o) -> p s o", o=O_DIM))
            rem = G - nfull * 128
            if rem:
                nc.sync.dma_start(out=out_d.ap()[nfull * 128:G, :],
                                  in_=onm[0:rem, nfull * O_DIM:(nfull + 1) * O_DIM])

    nc.compile()
    return nc


def _build_in_maps(inputs, dims, arrays):
    import ml_dtypes
    f = lambda x: np.ascontiguousarray(np.asarray(x, np.float32))
    col = lambda x: f(x).reshape(-1, 1)
    shared = {
        "iota": np.tile(np.arange(max(dims["GWIN"], 128), dtype=np.float32),
                        (128, 1)).astype(ml_dtypes.bfloat16),
        "W1a": f(inputs["W1a"]), "W1b": f(inputs["W1b"]),
        "W2a": f(inputs["W2a"]), "W2b": f(inputs["W2b"]),
        "Wf1": f(inputs["Wf1"]), "Wf2": f(inputs["Wf2"]),
        "Wc1": f(inputs["Wc1"]), "Wc2": f(inputs["Wc2"]),
        "b1a": col(inputs["b1a"]), "b1b": col(inputs["b1b"]),
        "b2a": col(inputs["b2a"]), "b2b": col(inputs["b2b"]),
        "bf1": col(inputs["bf1"]), "bf2": col(inputs["bf2"]),
        "bc1": col(inputs["bc1"]), "bc2": col(inputs["bc2"]),
        "gc": col(inputs["gc"]), "bec": col(inputs["bec"]),
        "rmc": col(inputs["rmc"]), "rvc": col(inputs["rvc"]),
        "a_prelu_v": np.full((HC_DIM, 1),
                             np.float32(np.asarray(inputs["a_prelu"]))),
    }
    for pfx in ["n1_", "n2_", "f1_", "f2_"]:
        for sfx in ["g", "b", "rm", "rv"]:
            shared[pfx + sfx] = col(inputs[pfx + sfx])
    in_maps = []
    for k in range(NCORES):
        m = dict(shared)
        m["posE"] = arrays["posE"][k].astype(ml_dtypes.bfloat16)
        m["pos_nm"] = arrays["pos_nm"][k].astype(ml_dtypes.bfloat16)
        m["batch_rel"] = arrays["batch_rel"][k]
        m["gidx2"] = arrays["gidx2"][k]
        m["dwc"] = arrays["dwc"][k]
        m["dwc2"] = arrays["dwc2"][k]
        m["groff"] = arrays["groff"][k]
        in_maps.append(m)
    return in_maps


def _get_compiled(pos, edge_index, batch, N, E, G):
    dims, arrays = _preprocess(pos, edge_index, batch, N, E, G)
    key = tuple(sorted((k, v) for k, v in dims.items()))
    if key not in _CACHE:
        _CACHE[key] = _build_program(dims)
    return _CACHE[key], dims, arrays


def kernel(**inputs):
    from concourse.bass_utils import run_bass_kernel_spmd
    pos = np.asarray(inputs["pos"])
    ei = np.asarray(inputs["edge_index"])
    batch = np.asarray(inputs["batch"])
    nc, dims, arrays = _get_compiled(pos, ei, batch, pos.shape[0],
                                     ei.shape[1], G_FULL)
    in_maps = _build_in_maps(inputs, dims, arrays)
    res = run_bass_kernel_spmd(nc, in_maps, list(range(NCORES)))
    return np.asarray(res.results[0]["out"], np.float32)

